# revision 4
# baseline (speedup 1.0000x reference)
"""GAT (2-layer, 4-head) + MLP/BatchNorm predictor on 8 Trainium2 NeuronCores.

v2 strategy (graph-parallel, dst-sharded; engine-balanced):
  - Nodes split contiguously: core c owns dsts [c*6250, (c+1)*6250). Edges live
    with their dst core, sorted by dst, grouped into 49 chunks of <=128 dsts.
  - Features are uploaded TRANSPOSED (featT [128, N] fp16) so the projection
    needs no on-chip transposes: per 128-row tile one fp16 matmul
    x @ [W | Wel | Wer] -> PSUM f32, cast f32->fp16 round-robin on
    DVE/Act/Pool, batched 4-tile DMA into the table (row = 256 h fp16
    (d,h)-interleaved | el 4xfp16 | er 4xfp16 | pad to 768B).
  - Edge phase per chunk: dma_gather of 768B src rows; dst-mask m_all built by
    one DVE is_equal (all-fp16 packed APs -> 2x mode); transpose-mask mt via
    PE transposes of m_all + one DVE copy; er broadcast dst->slot via tiny
    mt matmuls; e = el+er, LeakyReLU on DVE, exp on Act (only Act func in the
    phase -> no activation-table reloads) written straight into the row pad;
    h *= ex as ONE DVE multiply at 2x rate thanks to the (d,h) interleave;
    aggregation + softmax denominators via T accumulating mask matmuls on PE;
    node-space epilogue (1/s scaling, bias, ReLU, head-mean) on GpSimd
    scalar_tensor_tensor ops; output transposed on PE into SBUF-resident
    x2T/x3T slabs.
  - Collectives: AllGather of x2T (fp16, [64, 6250] per core) so each core
    projects the full layer-2 table; single AllReduce of BatchNorm raw
    moment sums (single-pass E[z], E[z^2] in f32).
"""
import sys

sys.path.insert(0, "/opt/trn_rl_repo")

import numpy as np

N = 50000
F_IN = 128
H = 4
D = 64
HD = 256
NCORES = 8
NSHARD = N // NCORES          # 6250
P = 128
NCHUNK = (NSHARD + P - 1) // P  # 49 (last chunk 106 dsts)
SPLIT = 32768                 # int16 gather index limit
MLP_H = 200
NCLS = 2
NEG = 0.2
EPS = 1e-5
ROW = 384                     # fp16 slots per table row (768 B)
ELOFF = 256                   # el at slots 256:260 (fp16), er at 260:264
EROFF = 260


def configure(n, split=32768):
    global N, NSHARD, NCHUNK, SPLIT
    N = n
    NSHARD = N // NCORES
    NCHUNK = (NSHARD + P - 1) // P
    SPLIT = split


# ----------------------------------------------------------------------------
# Host-side preprocessing
# ----------------------------------------------------------------------------

_PERM = None


def _perm():
    """Column permutation (h,d) -> (d,h): new[d*H+h] = old[h*D+d]."""
    global _PERM
    if _PERM is None:
        idx = np.arange(HD).reshape(H, D)          # old[h, d]
        _PERM = idx.T.reshape(-1)                  # new[(d, h)]
    return _PERM


def _fold_weights(W, al, ar):
    """W:[F,H*D] al,ar:[H,D] -> Wext [F, HD+8] f16 with (d,h) interleave."""
    F = W.shape[0]
    W64 = W.astype(np.float64)
    Wel = (W64.reshape(F, H, D) * al[None].astype(np.float64)).sum(-1)  # [F,H]
    Wer = (W64.reshape(F, H, D) * ar[None].astype(np.float64)).sum(-1)
    Wr = W64[:, _perm()]
    return np.concatenate([Wr, Wel, Wer], axis=1).astype(np.float16)


def _prep_edges(src, dst):
    """Per-core gather arrays. Returns (plan, per_core arrays).

    plan: T_lo[j], T_hi[j], totT, TMAX (identical across cores).
    per-core: IDX [128, 8*totT] i16, DLC [128, totT] f16 (pad -1).
    """
    src = np.asarray(src)
    dst = np.asarray(dst)
    per_core = []
    for c in range(NCORES):
        m = (dst >= c * NSHARD) & (dst < (c + 1) * NSHARD)
        es, ed = src[m], dst[m] - c * NSHARD
        order = np.argsort(ed, kind="stable")
        es, ed = es[order], ed[order]
        starts = np.searchsorted(ed, np.arange(0, NCHUNK * P, P))
        ends = np.searchsorted(ed, np.minimum(np.arange(P, (NCHUNK + 1) * P, P), NSHARD))
        chunks = []
        for j in range(NCHUNK):
            cs, ce = starts[j], ends[j]
            s_j, d_j = es[cs:ce], ed[cs:ce] - j * P
            lo = s_j < SPLIT
            # sort each segment by src: the dma_gather then reads ascending
            # HBM addresses (row-buffer locality); dst identity rides in dlc.
            slo, dlo = s_j[lo], d_j[lo]
            o = np.argsort(slo, kind="stable")
            shi, dhi = s_j[~lo] - SPLIT, d_j[~lo]
            o2 = np.argsort(shi, kind="stable")
            chunks.append((slo[o], dlo[o], shi[o2], dhi[o2]))
        per_core.append(chunks)

    T_lo = np.zeros(NCHUNK, np.int64)
    T_hi = np.zeros(NCHUNK, np.int64)
    for c in range(NCORES):
        for j in range(NCHUNK):
            slo, _, shi, _ = per_core[c][j]
            T_lo[j] = max(T_lo[j], -(-len(slo) // P))
            T_hi[j] = max(T_hi[j], -(-len(shi) // P))
    T_lo = np.maximum(T_lo, 1)
    totT = int((T_lo + T_hi).sum())
    TMAX = int((T_lo + T_hi).max())

    def wrap_idx(flat):
        n = len(flat)
        cols = n // 16
        a = flat.reshape(cols, 16).T.astype(np.int16)      # [16, cols]
        return np.tile(a, (8, 1))                          # [128, cols]

    arrays = []
    for c in range(NCORES):
        idx_cols = []
        dlc = np.full((P, totT), -1.0, np.float16)
        t0 = 0
        for j in range(NCHUNK):
            slo, dlo, shi, dhi = per_core[c][j]
            for (s_j, d_j, T) in ((slo, dlo, T_lo[j]), (shi, dhi, T_hi[j])):
                nslot = int(T) * P
                if nslot == 0:
                    continue
                idx = np.zeros(nslot, np.int16)
                dl = np.full(nslot, -1.0, np.float32)
                idx[: len(s_j)] = s_j
                dl[: len(s_j)] = d_j
                idx_cols.append(wrap_idx(idx))
                dlc[:, t0 : t0 + int(T)] = dl.reshape(int(T), P).T.astype(np.float16)
                t0 += int(T)
        assert t0 == totT
        IDX = np.concatenate(idx_cols, axis=1)
        assert IDX.shape == (P, 8 * totT)
        arrays.append((IDX, dlc))

    plan = {"T_lo": T_lo.tolist(), "T_hi": T_hi.tolist(), "totT": totT,
            "TMAX": TMAX}
    return plan, arrays


# ----------------------------------------------------------------------------
# Bass program
# ----------------------------------------------------------------------------

def build_nc(plan, phases='full', reps=1, max_chunks=None):
    import concourse.bacc as bacc
    import concourse.bass as bass
    import concourse.tile as tile
    from concourse import mybir

    FP16 = mybir.dt.float16
    F32 = mybir.dt.float32
    I16 = mybir.dt.int16
    ALU = mybir.AluOpType
    ACTF = mybir.ActivationFunctionType

    T_lo, T_hi, totT = plan["T_lo"], plan["T_hi"], plan["totT"]
    TMAX = plan["TMAX"]
    NTILE = (N + P - 1) // P            # 391 (last 80 rows)
    WCOLS = HD + 8

    nc = bacc.Bacc("TRN2", target_bir_lowering=False, debug=False,
                   num_devices=NCORES)

    dp = lambda name, shape, dt: nc.declare_dram_parameter(name, shape, dt, isOutput=False)
    FEATT = dp("FEATT", [P, N], FP16)
    FOWNT = dp("FOWNT", [P, NSHARD], FP16)
    IDX = dp("IDX", [P, 8 * totT], I16)
    DLC = dp("DLC", [P, totT], FP16)
    IOTAW = dp("IOTAW", [P, P * TMAX], FP16)
    IDENT = dp("IDENT", [P, P], FP16)
    IDENTF = dp("IDENTF", [P, P], F32)
    W1EXT = dp("W1EXT", [F_IN, WCOLS], FP16)
    W2EXT = dp("W2EXT", [D, WCOLS], FP16)
    B1Q = dp("B1Q", [P, HD], F32)
    B2Q = dp("B2Q", [P, HD], F32)
    WM1F = dp("WM1F", [D, MLP_H], F32)
    BM1 = dp("BM1", [P, MLP_H], F32)
    WM2C1F = dp("WM2C1F", [P, NCLS], F32)
    WM2C2F = dp("WM2C2F", [MLP_H - P, NCLS], F32)
    GB = dp("GB", [P, 4], F32)
    BM2 = dp("BM2", [1, NCLS], F32)
    ONESF = dp("ONESF", [P, 1], F32)

    out = nc.declare_dram_parameter("out", [NSHARD, NCLS], F32, isOutput=True)

    table1 = nc.dram_tensor("table1", [N, ROW], FP16)
    table2 = nc.dram_tensor("table2", [N, ROW], FP16)
    AGSPL = 25 * P                       # AllGather stage split (3200)
    x2sliceA = nc.dram_tensor("x2sliceA", [D, AGSPL], FP16)
    x2sliceB = nc.dram_tensor("x2sliceB", [D, NSHARD - AGSPL], FP16)
    x2fullA = nc.dram_tensor("x2fullA", [NCORES * D, AGSPL], FP16,
                             addr_space="Shared")
    x2fullB = nc.dram_tensor("x2fullB", [NCORES * D, NSHARD - AGSPL], FP16,
                             addr_space="Shared")
    ccin = nc.dram_tensor("ccin", [P, 4], F32)
    ccout = nc.dram_tensor("ccout", [P, 4], F32, addr_space="Shared")

    class _SkipRest(Exception):
        pass

    with tile.TileContext(nc) as tc:
        import contextlib
        try:
          with contextlib.ExitStack() as ctx:
            singles = ctx.enter_context(tc.tile_pool(name="singles", bufs=1))

            def load_const(param, shape, dtype, tag):
                t = singles.tile(shape, dtype, tag=tag)
                nc.sync.dma_start(out=t[:], in_=param[:])
                return t

            identb = load_const(IDENT, [P, P], FP16, "c_ident")
            identf = load_const(IDENTF, [P, P], F32, "c_identf")
            iotaw = load_const(IOTAW, [P, P, TMAX], FP16, "c_iotaw")
            w1ext = load_const(W1EXT, [F_IN, WCOLS], FP16, "c_w1ext")
            w2ext = load_const(W2EXT, [D, WCOLS], FP16, "c_w2ext")
            b1q = load_const(B1Q, [P, HD], F32, "c_b1q")
            b2q = load_const(B2Q, [P, HD], F32, "c_b2q")
            wm1f = load_const(WM1F, [D, MLP_H], F32, "c_wm1f")
            bm1 = load_const(BM1, [P, MLP_H], F32, "c_bm1")
            wm2c1f = load_const(WM2C1F, [P, NCLS], F32, "c_wm2c1f")
            wm2c2f = load_const(WM2C2F, [MLP_H - P, NCLS], F32, "c_wm2c2f")
            gb = load_const(GB, [P, 4], F32, "c_gb")
            bm2 = load_const(BM2, [1, NCLS], F32, "c_bm2")
            onesf = load_const(ONESF, [P, 1], F32, "c_onesf")
            fownt = load_const(FOWNT, [P, NSHARD], FP16, "c_fownt")
            idx_sb = load_const(IDX, [P, 8 * totT], I16, "c_idx")
            dlc_sb = load_const(DLC, [P, totT], FP16, "c_dlc")

            x2t_sb = singles.tile([D, NSHARD], FP16, tag="c_x2t")
            x3t_sb = singles.tile([D, NSHARD], F32, tag="c_x3t")
            erown1 = singles.tile([P, NCHUNK, 4], FP16, tag="c_erown1")
            erown2 = singles.tile([P, NCHUNK, 4], FP16, tag="c_erown2")
            zstore = singles.tile([P, NCHUNK, MLP_H + 1], F32, tag="c_zstore")

            def _run_once():
                nc.vector.memset(erown1[:], 0.0)
                nc.vector.memset(erown2[:], 0.0)
                nc.vector.memset(zstore[:], 0.0)
                # ones column (col MLP_H) for the pass-D folded-constant row
                nc.vector.memset(zstore[:, :, MLP_H:MLP_H + 1], 1.0)

                # ---------------- projection phase (full table) --------------
                def projection(layer):
                    """layer 1: featT -> table1; layer 2: x2fullT -> table2."""
                    F = F_IN if layer == 1 else D
                    wext = w1ext if layer == 1 else w2ext
                    table = table1 if layer == 1 else table2
                    SLAB = 8                     # tiles per load slab
                    BST = 8                      # tiles per store batch
                    with tc.tile_pool(name="proj_sb", bufs=3) as sb, \
                         tc.tile_pool(name="proj_st", bufs=2) as stp, \
                         tc.tile_pool(name="proj_ps", bufs=4, space="PSUM") as ps:
                        nslab = (NTILE + SLAB - 1) // SLAB
                        cast_i = 0
                        slab_order = list(range(nslab))
                        if layer == 2:
                            # A-only slabs first: they depend only on the
                            # stage-A AllGather and overlap the in-flight
                            # stage-B collective.
                            def _a_only(s):
                                r0 = s * SLAB * P
                                hi = min(r0 + SLAB * P, N)
                                for r in (r0, hi - 1):
                                    if (r % NSHARD) >= AGSPL:
                                        return False
                                return (r0 // NSHARD) == ((hi - 1) // NSHARD)
                            slab_order.sort(key=lambda s: not _a_only(s))
                        for s in slab_order:
                            r0 = s * SLAB * P
                            ncols = min(SLAB * P, N - r0)
                            slab = sb.tile([F, SLAB * P], FP16, tag="slab")
                            if layer == 1:
                                nc.sync.dma_start(out=slab[:, 0:ncols],
                                                  in_=FEATT[:, r0:r0 + ncols])
                            else:
                                # x2full rows live in per-core 64-row bands,
                                # split at AGSPL into the A/B staged tensors
                                lo = r0
                                while lo < r0 + ncols:
                                    c = lo // NSHARD
                                    lc = lo - c * NSHARD
                                    if lc < AGSPL:
                                        hi = min(r0 + ncols,
                                                 c * NSHARD + AGSPL)
                                        srct, off = x2fullA, lc
                                    else:
                                        hi = min(r0 + ncols, (c + 1) * NSHARD)
                                        srct, off = x2fullB, lc - AGSPL
                                    nc.sync.dma_start(
                                        out=slab[:, lo - r0:hi - r0],
                                        in_=srct[c * D:(c + 1) * D,
                                                 off:off + (hi - lo)])
                                    lo = hi
                            ntile_s = (ncols + P - 1) // P
                            for b0 in range(0, ntile_s, BST):
                                nb = min(BST, ntile_s - b0)
                                rowt = stp.tile([P, BST, WCOLS], FP16, tag="rowt")
                                for q in range(nb):
                                    k = b0 + q
                                    rows = min(P, ncols - k * P)
                                    hp = ps.tile([P, WCOLS], F32, tag="hp")
                                    nc.tensor.matmul(hp[:rows, :],
                                                     lhsT=slab[:, k * P:k * P + rows],
                                                     rhs=wext[:],
                                                     start=True, stop=True)
                                    # GPSIMD cannot read PSUM: rotate
                                    # 1:2 DVE:Act (DVE is the busier engine)
                                    eng = (nc.vector, nc.scalar,
                                           nc.scalar)[cast_i % 3]
                                    cast_i += 1
                                    if eng is nc.scalar:
                                        nc.scalar.activation(rowt[:rows, q, :],
                                                             hp[:rows, :], ACTF.Copy)
                                    else:
                                        nc.vector.tensor_copy(out=rowt[:rows, q, :],
                                                              in_=hp[:rows, :])
                                rows_b = min(BST * P, ncols - b0 * P)
                                nfull = rows_b // P
                                dst_r0 = r0 + b0 * P
                                if nfull:
                                    trows = table[dst_r0:dst_r0 + nfull * P, 0:WCOLS]
                                    nc.sync.dma_start(
                                        out=trows.rearrange("(q p) c -> p q c", p=P),
                                        in_=rowt[:, 0:nfull, :])
                                tail = rows_b - nfull * P
                                if tail:
                                    nc.sync.dma_start(
                                        out=table[dst_r0 + nfull * P:
                                                  dst_r0 + rows_b, 0:WCOLS],
                                        in_=rowt[:tail, nfull, :])

                # --------------- own-er prologue (per-chunk er) --------------
                def er_prologue(xt_src, wext, dest):
                    with tc.tile_pool(name="er_sb", bufs=2) as sb, \
                         tc.tile_pool(name="er_ps", bufs=1, space="PSUM") as ps:
                        erp_all = ps.tile([P, NCHUNK, 4], F32, tag="erp_all")
                        for j in range(NCHUNK):
                            rows = min(P, NSHARD - j * P)
                            nc.tensor.matmul(erp_all[:rows, j, :],
                                             lhsT=xt_src[:, j * P:j * P + rows],
                                             rhs=wext[:, WCOLS - 4:WCOLS],
                                             start=True, stop=True)
                        nfull = NCHUNK - 1
                        nc.vector.tensor_copy(out=dest[:, 0:nfull, :],
                                              in_=erp_all[:, 0:nfull, :])
                        lrows = NSHARD - (NCHUNK - 1) * P
                        nc.vector.tensor_copy(out=dest[:lrows, nfull, :],
                                              in_=erp_all[:lrows, nfull, :])

                # ------------------------- edge phase ------------------------
                def edge_phase(table, ero, bias_c, layer):
                    nch = NCHUNK if max_chunks is None else min(max_chunks, NCHUNK)
                    with tc.tile_pool(name="eg", bufs=3) as eg, \
                         tc.tile_pool(name="em", bufs=2) as em, \
                         tc.tile_pool(name="emt", bufs=2) as emt, \
                         tc.tile_pool(name="es", bufs=3) as es_pool, \
                         tc.tile_pool(name="eps", bufs=1, space="PSUM") as eps, \
                         tc.tile_pool(name="epa", bufs=2, space="PSUM") as epa, \
                         tc.tile_pool(name="epe", bufs=2, space="PSUM") as epe, \
                         tc.tile_pool(name="epx", bufs=1, space="PSUM") as epx:
                        toff = 0
                        for j in range(nch):
                            Tl, Th = T_lo[j], T_hi[j]
                            T = Tl + Th
                            rows = min(P, NSHARD - j * P)
                            gbuf = eg.tile([P, TMAX, ROW], FP16, tag="gbuf")
                            nc.gpsimd.dma_gather(
                                out_ap=gbuf[:, 0:Tl, :], in_ap=table[0:SPLIT, :],
                                idxs_ap=idx_sb[:, 8 * toff:8 * (toff + Tl)],
                                num_idxs=P * Tl, num_idxs_reg=P * Tl,
                                elem_size=ROW, single_packet=False)
                            if Th:
                                nc.gpsimd.dma_gather(
                                    out_ap=gbuf[:, Tl:T, :], in_ap=table[SPLIT:N, :],
                                    idxs_ap=idx_sb[:, 8 * (toff + Tl):8 * (toff + T)],
                                    num_idxs=P * Th, num_idxs_reg=P * Th,
                                    elem_size=ROW, single_packet=False)
                            # dst mask m_all[p, c, t] = (c == dlc[p, t])
                            m_all = em.tile([P, P, TMAX], FP16, tag="m_all")
                            dsl = dlc_sb[:, toff:toff + T]
                            dlc_b = bass.AP(tensor=dsl.tensor, offset=dsl.offset,
                                            ap=[dsl.ap[0], [0, P]] + dsl.ap[1:])
                            nc.vector.tensor_tensor(out=m_all[:, :, 0:T],
                                                    in0=iotaw[:, :, 0:T],
                                                    in1=dlc_b, op=ALU.is_equal)
                            # mt = transpose(m_all) per tile, via PE + one copy
                            mtp = eps.tile([P, TMAX, P], FP16, tag="mtp")
                            for t in range(T):
                                nc.tensor.transpose(out=mtp[:, t, :],
                                                    in_=m_all[:, :, t],
                                                    identity=identb[:])
                            mt = emt.tile([P, TMAX, P], FP16, tag="mt")
                            nc.scalar.activation(mt[:, 0:T, :], mtp[:, 0:T, :],
                                                 ACTF.Copy)
                            # er per slot: erp[p, t, :] = mt_t^T @ ero
                            erp = epe.tile([P, TMAX, 4], F32, tag="erp")
                            for t in range(T):
                                nc.tensor.matmul(erp[:, t, :], lhsT=mt[:, t, :],
                                                 rhs=ero[:, j, :],
                                                 start=True, stop=True)
                            # e = el + er ; lrelu ; exp -> gbuf[...,260:264]
                            e_sb = es_pool.tile([P, TMAX, 4], F32, tag="e_sb")
                            nc.vector.tensor_tensor(out=e_sb[:, 0:T, :],
                                                    in0=gbuf[:, 0:T, ELOFF:ELOFF + 4],
                                                    in1=erp[:, 0:T, :], op=ALU.add)
                            lr = es_pool.tile([P, TMAX, 4], F32, tag="lr")
                            nc.scalar.activation(lr[:, 0:T, :], e_sb[:, 0:T, :],
                                                 ACTF.Prelu, alpha=NEG)
                            gex = gbuf[:, 0, EROFF:EROFF + 4]
                            ex_out = bass.AP(tensor=gex.tensor, offset=gex.offset,
                                             ap=[gex.ap[0], [ROW, T], [1, 4]])
                            nc.scalar.activation(ex_out, lr[:, 0:T, :], ACTF.Exp)
                            # h *= ex (2x-rate thanks to (d,h) interleave)
                            gb0 = gbuf[:, 0, 0:HD]
                            hv = bass.AP(tensor=gb0.tensor, offset=gb0.offset,
                                         ap=[gb0.ap[0], [ROW, T], [H, D], [1, H]])
                            ex_b = bass.AP(tensor=gex.tensor, offset=gex.offset,
                                           ap=[gex.ap[0], [ROW, T], [0, D], [1, H]])
                            nc.vector.tensor_tensor(out=hv, in0=hv, in1=ex_b,
                                                    op=ALU.mult)
                            # aggregate: T accumulating mask matmuls
                            agg = epa.tile([P, WCOLS], F32, tag="agg")
                            for t in range(T):
                                nc.tensor.matmul(agg[:], lhsT=m_all[:, :, t],
                                                 rhs=gbuf[:, t, 0:WCOLS],
                                                 start=(t == 0), stop=(t == T - 1))
                            # node-space epilogue on DVE(recip) + GpSimd
                            sr = es_pool.tile([P, 4], F32, tag="sr")
                            nc.vector.reciprocal(sr[:], agg[:, EROFF:EROFF + 4])
                            agg_r = agg[:, 0:HD].rearrange("p (d h) -> p d h", h=H)
                            sr_ap = sr[:]
                            sr_b = bass.AP(tensor=sr_ap.tensor, offset=sr_ap.offset,
                                           ap=[sr_ap.ap[0], [0, D], [1, H]])
                            osb = es_pool.tile([P, D, H], F32, tag="osb")
                            # agg is PSUM: this one stays on DVE (GPSIMD
                            # cannot read PSUM); the rest go to GpSimd.
                            nc.vector.tensor_tensor(out=osb[:], in0=agg_r,
                                                    in1=sr_b, op=ALU.mult)
                            bias_r = bias_c[:].rearrange("p (d h) -> p d h", h=H)
                            nc.vector.tensor_tensor(out=osb[:], in0=osb[:],
                                                    in1=bias_r, op=ALU.add)
                            # 0.25*relu(x) == relu(0.25*x): head-mean scale
                            # folded into the Act scale.
                            nc.scalar.activation(osb[:], osb[:], ACTF.Relu,
                                                 scale=0.25)
                            xo = es_pool.tile([P, D], F32, tag="xo")
                            nc.vector.tensor_reduce(
                                out=xo[:], in_=osb[:],
                                axis=mybir.AxisListType.X, op=ALU.add)
                            # transpose -> [D, rows] and store into xT slab
                            xop = epx.tile([D, P], F32, tag="xop")
                            nc.tensor.transpose(out=xop[:, 0:rows],
                                                in_=xo[:rows, :],
                                                identity=identf[:rows, :rows])
                            if layer == 1:
                                nc.scalar.activation(
                                    x2t_sb[:, j * P:j * P + rows],
                                    xop[:, 0:rows], ACTF.Copy)
                            else:
                                nc.scalar.activation(
                                    x3t_sb[:, j * P:j * P + rows],
                                    xop[:, 0:rows], ACTF.Copy)
                            if layer == 1 and j == 24:
                                nc.sync.dma_start(out=x2sliceA[:],
                                                  in_=x2t_sb[:, 0:AGSPL])
                                nc.gpsimd.collective_compute(
                                    "AllGather", mybir.AluOpType.bypass,
                                    replica_groups=[list(range(NCORES))],
                                    ins=[x2sliceA[:]], outs=[x2fullA[:]])
                            toff += T

                # ------------------------------ go ---------------------------
                order = ["P1", "E1", "AG", "P2", "E2", "full"]
                upto = order.index(phases)
                done = False

                projection(1)
                er_prologue(fownt, w1ext, erown1)
                done = upto <= order.index("P1")
                if not done:
                    edge_phase(table1, erown1, b1q, layer=1)
                    nc.sync.dma_start(out=x2sliceB[:],
                                      in_=x2t_sb[:, AGSPL:NSHARD])
                    done = upto <= order.index("E1")
                if not done:
                    nc.gpsimd.collective_compute(
                        "AllGather", mybir.AluOpType.bypass,
                        replica_groups=[list(range(NCORES))],
                        ins=[x2sliceB[:]], outs=[x2fullB[:]])
                    done = upto <= order.index("AG")
                if not done:
                    projection(2)
                    er_prologue(x2t_sb, w2ext, erown2)
                    done = upto <= order.index("P2")
                if not done:
                    edge_phase(table2, erown2, b2q, layer=2)
                    done = upto <= order.index("E2")
                if done:
                    with tc.tile_pool(name="dbg0", bufs=1) as dbg0:
                        z = dbg0.tile([P, NCLS], F32, tag="dbgz")
                        nc.vector.memset(z[:], 0.0)
                        for j in range(NCHUNK):
                            r0 = j * P
                            rows = min(P, NSHARD - r0)
                            nc.sync.dma_start(out=out[r0:r0 + rows, :], in_=z[:rows])
                    raise _SkipRest()

                # ------------------------------ MLP --------------------------
                # pass A: z = relu(x3 @ Wm1 + bm1) -> zstore; raw moment sums
                with tc.tile_pool(name="ma", bufs=3) as ma, \
                     tc.tile_pool(name="map", bufs=2, space="PSUM") as map_, \
                     tc.tile_pool(name="sta", bufs=1, space="PSUM") as sta:
                    sa1 = sta.tile([P, 1], F32, tag="sa1")
                    sa2 = sta.tile([P, 1], F32, tag="sa2")
                    sq1 = sta.tile([P, 1], F32, tag="sq1")
                    sq2 = sta.tile([P, 1], F32, tag="sq2")
                    for j in range(NCHUNK):
                        rows = min(P, NSHARD - j * P)
                        zp = map_.tile([P, MLP_H], F32, tag="zp")
                        nc.tensor.matmul(zp[:rows, :],
                                         lhsT=x3t_sb[:, j * P:j * P + rows],
                                         rhs=wm1f[:], start=True, stop=True)
                        zc = zstore[:, j, 0:MLP_H]
                        nc.vector.tensor_tensor(out=zc[:rows], in0=zp[:rows],
                                                in1=bm1[:rows], op=ALU.add)
                        nc.scalar.activation(zc[:rows], zc[:rows], ACTF.Relu)
                        zq = ma.tile([P, MLP_H], F32, tag="zq")
                        nc.vector.tensor_tensor(out=zq[:rows], in0=zc[:rows],
                                                in1=zc[:rows], op=ALU.mult)
                        first, last = (j == 0), (j == NCHUNK - 1)
                        nc.tensor.matmul(sa1[:], lhsT=zc[:rows, 0:P],
                                         rhs=onesf[:rows], start=first, stop=last)
                        nc.tensor.matmul(sa2[:MLP_H - P], lhsT=zc[:rows, P:MLP_H],
                                         rhs=onesf[:rows], start=first, stop=last)
                        nc.tensor.matmul(sq1[:], lhsT=zq[:rows, 0:P],
                                         rhs=onesf[:rows], start=first, stop=last)
                        nc.tensor.matmul(sq2[:MLP_H - P], lhsT=zq[:rows, P:MLP_H],
                                         rhs=onesf[:rows], start=first, stop=last)
                    pk = ma.tile([P, 4], F32, tag="pk")
                    nc.vector.memset(pk[:], 0.0)
                    nc.vector.tensor_copy(out=pk[:, 0:1], in_=sa1[:])
                    nc.vector.tensor_copy(out=pk[:MLP_H - P, 1:2], in_=sa2[:MLP_H - P])
                    nc.vector.tensor_copy(out=pk[:, 2:3], in_=sq1[:])
                    nc.vector.tensor_copy(out=pk[:MLP_H - P, 3:4], in_=sq2[:MLP_H - P])
                    nc.sync.dma_start(out=ccin[:], in_=pk[:])

                nc.gpsimd.collective_compute(
                    "AllReduce", mybir.AluOpType.add,
                    replica_groups=[list(range(NCORES))],
                    ins=[ccin[:]], outs=[ccout[:]])

                # pass C: BN constants folded into final weights
                with tc.tile_pool(name="m2", bufs=3) as m2, \
                     tc.tile_pool(name="m2p", bufs=2, space="PSUM") as m2p:
                    stg = m2.tile([P, 4], F32, tag="stg")
                    nc.sync.dma_start(out=stg[:], in_=ccout[:])
                    m1t = m2.tile([P, 2], F32, tag="m1t")
                    nc.vector.tensor_scalar(out=m1t[:], in0=stg[:, 0:2],
                                            scalar1=1.0 / N, scalar2=None,
                                            op0=ALU.mult)
                    m2t = m2.tile([P, 2], F32, tag="m2t")
                    nc.vector.tensor_scalar(out=m2t[:], in0=stg[:, 2:4],
                                            scalar1=1.0 / N, scalar2=None,
                                            op0=ALU.mult)
                    var = m2.tile([P, 2], F32, tag="var")
                    nc.vector.tensor_tensor(out=var[:], in0=m1t[:], in1=m1t[:],
                                            op=ALU.mult)
                    nc.vector.tensor_tensor(out=var[:], in0=m2t[:], in1=var[:],
                                            op=ALU.subtract)
                    nc.vector.tensor_scalar(out=var[:], in0=var[:], scalar1=EPS,
                                            scalar2=None, op0=ALU.add)
                    std = m2.tile([P, 2], F32, tag="std")
                    nc.scalar.activation(std[:], var[:], ACTF.Sqrt)
                    rstd = m2.tile([P, 2], F32, tag="rstd")
                    nc.vector.reciprocal(rstd[:], std[:])
                    gp = m2.tile([P, 2], F32, tag="gp")
                    nc.vector.tensor_tensor(out=gp[:], in0=gb[:, 0:2], in1=rstd[:],
                                            op=ALU.mult)
                    bp = m2.tile([P, 2], F32, tag="bp")
                    nc.vector.tensor_tensor(out=bp[:], in0=m1t[:], in1=gp[:],
                                            op=ALU.mult)
                    nc.vector.tensor_tensor(out=bp[:], in0=gb[:, 2:4], in1=bp[:],
                                            op=ALU.subtract)
                    wp1 = m2.tile([P, NCLS], F32, tag="wp1")
                    nc.vector.tensor_scalar_mul(wp1[:], wm2c1f[:], gp[:, 0:1])
                    wp2 = m2.tile([P, NCLS], F32, tag="wp2")
                    nc.vector.memset(wp2[:], 0.0)
                    nc.vector.tensor_scalar_mul(wp2[:MLP_H - P, :], wm2c2f[:],
                                                gp[:MLP_H - P, 1:2])
                    cp = m2p.tile([1, NCLS], F32, tag="cp")
                    nc.tensor.matmul(cp[:], lhsT=bp[:, 0:1], rhs=wm2c1f[:],
                                     start=True, stop=False)
                    nc.tensor.matmul(cp[:], lhsT=bp[:MLP_H - P, 1:2], rhs=wm2c2f[:],
                                     start=False, stop=True)
                    cps = m2.tile([1, NCLS], F32, tag="cps")
                    nc.vector.tensor_tensor(out=cps[:], in0=cp[:], in1=bm2[:],
                                            op=ALU.add)
                    # place c'' into wp2 row (MLP_H - P) — pairs with the
                    # all-ones zstore column MLP_H. DMA: cross-partition move.
                    nc.sync.dma_start(out=wp2[MLP_H - P:MLP_H - P + 1, :],
                                      in_=cps[:])

                    # pass D: out = z @ W'' (+ c'' via ones column)
                    for j in range(NCHUNK):
                        r0 = j * P
                        rows = min(P, NSHARD - r0)
                        t1 = m2p.tile([P, P], F32, tag="t1")
                        nc.tensor.transpose(out=t1[:], in_=zstore[:, j, 0:P],
                                            identity=identf[:])
                        zT1 = m2.tile([P, P], F32, tag="zT1")
                        nc.scalar.activation(zT1[:], t1[:], ACTF.Copy)
                        t2 = m2p.tile([MLP_H + 1 - P, P], F32, tag="t2")
                        nc.tensor.transpose(out=t2[:], in_=zstore[:, j, P:MLP_H + 1],
                                            identity=identf[:])
                        zT2 = m2.tile([MLP_H + 1 - P, P], F32, tag="zT2")
                        nc.scalar.activation(zT2[:], t2[:], ACTF.Copy)
                        op_ = m2p.tile([P, NCLS], F32, tag="op")
                        nc.tensor.matmul(op_[:], lhsT=zT1[:], rhs=wp1[:],
                                         start=True, stop=False)
                        nc.tensor.matmul(op_[:], lhsT=zT2[:], rhs=wp2[:MLP_H + 1 - P, :],
                                         start=False, stop=True)
                        ofin = m2.tile([P, NCLS], F32, tag="ofin")
                        nc.vector.tensor_copy(out=ofin[:rows], in_=op_[:rows])
                        nc.sync.dma_start(out=out[r0:r0 + rows, :], in_=ofin[:rows])

            for _rep in range(reps):
                try:
                    _run_once()
                except _SkipRest:
                    pass

        except _SkipRest:
            pass
    nc.finalize()
    return nc


# ----------------------------------------------------------------------------
# Host entry
# ----------------------------------------------------------------------------

def prep_inputs(inputs):
    f32 = np.float32
    perm = _perm()
    W1ext = _fold_weights(np.asarray(inputs["W1"], f32),
                          np.asarray(inputs["al1"], f32),
                          np.asarray(inputs["ar1"], f32))
    W2ext = _fold_weights(np.asarray(inputs["W2"], f32),
                          np.asarray(inputs["al2"], f32),
                          np.asarray(inputs["ar2"], f32))
    plan, earrays = _prep_edges(inputs["src"], inputs["dst"])
    TMAX = plan["TMAX"]

    featT = np.ascontiguousarray(
        np.asarray(inputs["features"], f32).astype(np.float16).T)  # [128, N]
    iotaw = np.broadcast_to(
        np.repeat(np.arange(P, dtype=np.float16), TMAX)[None, :],
        (P, P * TMAX)).copy()
    bc = lambda v: np.broadcast_to(np.asarray(v, f32).reshape(1, -1),
                                   (P, np.asarray(v).size)).copy()
    consts = {
        "IDENT": np.eye(P, dtype=np.float16),
        "IDENTF": np.eye(P, dtype=f32),
        "IOTAW": iotaw,
        "W1EXT": W1ext, "W2EXT": W2ext,
        "B1Q": bc(np.asarray(inputs["b1"], f32)[perm]),
        "B2Q": bc(np.asarray(inputs["b2"], f32)[perm]),
        "WM1F": np.asarray(inputs["Wm1"], f32),
        "BM1": bc(inputs["bm1"]),
        "WM2C1F": np.asarray(inputs["Wm2"], f32)[0:P, :],
        "WM2C2F": np.asarray(inputs["Wm2"], f32)[P:MLP_H, :],
        "BM2": np.asarray(inputs["bm2"], f32).reshape(1, NCLS),
        "ONESF": np.ones((P, 1), f32),
    }
    gbarr = np.zeros((P, 4), f32)
    gamma = np.asarray(inputs["gamma"], f32)
    beta = np.asarray(inputs["beta"], f32)
    gbarr[:, 0] = gamma[0:P]
    gbarr[0:MLP_H - P, 1] = gamma[P:MLP_H]
    gbarr[:, 2] = beta[0:P]
    gbarr[0:MLP_H - P, 3] = beta[P:MLP_H]
    consts["GB"] = gbarr

    in_maps = []
    for c in range(NCORES):
        IDXa, DLCa = earrays[c]
        m = dict(consts)
        m["FEATT"] = featT
        m["FOWNT"] = np.ascontiguousarray(featT[:, c * NSHARD:(c + 1) * NSHARD])
        m["IDX"] = IDXa
        m["DLC"] = DLCa
        in_maps.append(m)
    return plan, in_maps


def kernel(**inputs):
    from concourse.bass_utils import run_bass_kernel_spmd
    plan, in_maps = prep_inputs(inputs)
    nc = build_nc(plan)
    res = run_bass_kernel_spmd(nc, in_maps, core_ids=list(range(NCORES)))
    out = np.concatenate([res.results[c]["out"] for c in range(NCORES)], axis=0)
    return out.astype(np.float32)


if __name__ == "__main__":
    import time
    t0 = time.time()
    plan, _ = _prep_edges(
        np.concatenate([np.random.randint(0, N, 800000), np.arange(N)]).astype(np.int32),
        np.concatenate([np.random.randint(0, N, 800000), np.arange(N)]).astype(np.int32))
    print("edge prep:", time.time() - t0, "totT:", plan["totT"], "TMAX:", plan["TMAX"])
    t0 = time.time()
    nc = build_nc(plan)
    print("build:", time.time() - t0)


# revision 5
# speedup vs baseline: 1.6766x; 1.6766x over previous
"""GAT (2-layer, 4-head) + MLP/BatchNorm predictor on 8 Trainium2 NeuronCores.

v2 strategy (graph-parallel, dst-sharded; engine-balanced):
  - Nodes split contiguously: core c owns dsts [c*6250, (c+1)*6250). Edges live
    with their dst core, sorted by dst, grouped into 49 chunks of <=128 dsts.
  - Features are uploaded TRANSPOSED (featT [128, N] fp16) so the projection
    needs no on-chip transposes: per 128-row tile one fp16 matmul
    x @ [W | Wel | Wer] -> PSUM f32, cast f32->fp16 round-robin on
    DVE/Act/Pool, batched 4-tile DMA into the table (row = 256 h fp16
    (d,h)-interleaved | el 4xfp16 | er 4xfp16 | pad to 768B).
  - Edge phase per chunk: dma_gather of 768B src rows; dst-mask m_all built by
    one DVE is_equal (all-fp16 packed APs -> 2x mode); transpose-mask mt via
    PE transposes of m_all + one DVE copy; er broadcast dst->slot via tiny
    mt matmuls; e = el+er, LeakyReLU on DVE, exp on Act (only Act func in the
    phase -> no activation-table reloads) written straight into the row pad;
    h *= ex as ONE DVE multiply at 2x rate thanks to the (d,h) interleave;
    aggregation + softmax denominators via T accumulating mask matmuls on PE;
    node-space epilogue (1/s scaling, bias, ReLU, head-mean) on GpSimd
    scalar_tensor_tensor ops; output transposed on PE into SBUF-resident
    x2T/x3T slabs.
  - Collectives: AllGather of x2T (fp16, [64, 6250] per core) so each core
    projects the full layer-2 table; single AllReduce of BatchNorm raw
    moment sums (single-pass E[z], E[z^2] in f32).
"""
import sys

sys.path.insert(0, "/opt/trn_rl_repo")

import numpy as np

N = 50000
F_IN = 128
H = 4
D = 64
HD = 256
NCORES = 8
NSHARD = N // NCORES          # 6250
P = 128
NCHUNK = (NSHARD + P - 1) // P  # 49 (last chunk 106 dsts)
SPLIT = 32768                 # int16 gather index limit
MLP_H = 200
NCLS = 2
NEG = 0.2
EPS = 1e-5
ROW = 384                     # fp16 slots per table row (768 B)
ELOFF = 256                   # el at slots 256:260 (fp16), er at 260:264
EROFF = 260


def configure(n, split=32768):
    global N, NSHARD, NCHUNK, SPLIT
    N = n
    NSHARD = N // NCORES
    NCHUNK = (NSHARD + P - 1) // P
    SPLIT = split


# ----------------------------------------------------------------------------
# Host-side preprocessing
# ----------------------------------------------------------------------------

_PERM = None


def _perm():
    """Column permutation (h,d) -> (d,h): new[d*H+h] = old[h*D+d]."""
    global _PERM
    if _PERM is None:
        idx = np.arange(HD).reshape(H, D)          # old[h, d]
        _PERM = idx.T.reshape(-1)                  # new[(d, h)]
    return _PERM


def _fold_weights(W, al, ar):
    """W:[F,H*D] al,ar:[H,D] -> Wext [F, HD+8] f16 with (d,h) interleave."""
    F = W.shape[0]
    W64 = W.astype(np.float64)
    Wel = (W64.reshape(F, H, D) * al[None].astype(np.float64)).sum(-1)  # [F,H]
    Wer = (W64.reshape(F, H, D) * ar[None].astype(np.float64)).sum(-1)
    Wr = W64[:, _perm()]
    return np.concatenate([Wr, Wel, Wer], axis=1).astype(np.float16)


def _prep_edges(src, dst):
    """Per-core gather arrays. Returns (plan, per_core arrays).

    plan: T_lo[j], T_hi[j], totT, TMAX (identical across cores).
    per-core: IDX [128, 8*totT] i16, DLC [128, totT] f16 (pad -1).
    """
    src = np.asarray(src)
    dst = np.asarray(dst)
    HIBASE = N - SPLIT                    # hi window covers [HIBASE, N)
    raw = []                              # (c, j) -> (s_sorted_by_class,)
    for c in range(NCORES):
        m = (dst >= c * NSHARD) & (dst < (c + 1) * NSHARD)
        es, ed = src[m], dst[m] - c * NSHARD
        order = np.argsort(ed, kind="stable")
        es, ed = es[order], ed[order]
        starts = np.searchsorted(ed, np.arange(0, NCHUNK * P, P))
        ends = np.searchsorted(ed, np.minimum(np.arange(P, (NCHUNK + 1) * P, P), NSHARD))
        chunks = []
        for j in range(NCHUNK):
            cs, ce = starts[j], ends[j]
            s_j, d_j = es[cs:ce], ed[cs:ce] - j * P
            # classes: 0 = lo-only (<HIBASE), 1 = flexible, 2 = hi-only
            cls = (s_j >= HIBASE).astype(np.int8) + (s_j >= SPLIT)
            o = np.argsort(cls, kind="stable")
            chunks.append((s_j[o], d_j[o], int((cls == 0).sum()),
                           int((cls == 1).sum())))
        raw.append(chunks)

    # pick (T_lo, T_hi) per chunk minimizing T_lo+T_hi; flexible edges
    # (src in [HIBASE, SPLIT)) can go to either window.
    T_lo = np.zeros(NCHUNK, np.int64)
    T_hi = np.zeros(NCHUNK, np.int64)
    for j in range(NCHUNK):
        best = None
        mmax = max(len(raw[c][j][0]) for c in range(NCORES))
        for tot in range(-(-mmax // P), 4 * (-(-mmax // P)) + 2):
            for tl in range(1, tot + 1):
                th = tot - tl
                ok = True
                for c in range(NCORES):
                    s_j, _, nlo, nflex = raw[c][j]
                    mc = len(s_j)
                    a_min = max(nlo, mc - th * P)
                    a_max = min(nlo + nflex, tl * P)
                    if a_min > a_max:
                        ok = False
                        break
                if ok:
                    best = (tl, th)
                    break
            if best:
                break
        assert best is not None
        T_lo[j], T_hi[j] = best
    totT = int((T_lo + T_hi).sum())
    TMAX = int((T_lo + T_hi).max())

    per_core = []
    for c in range(NCORES):
        chunks = []
        for j in range(NCHUNK):
            s_j, d_j, nlo, nflex = raw[c][j]
            mc = len(s_j)
            a = max(nlo, mc - int(T_hi[j]) * P)
            a = min(max(a, 0), min(nlo + nflex, int(T_lo[j]) * P))
            # first a edges (classes 0 then 1, stably ordered) -> lo window
            slo, dlo = s_j[:a], d_j[:a]
            shi, dhi = s_j[a:] - HIBASE, d_j[a:]
            assert slo.size == 0 or slo.max() < SPLIT
            assert shi.size == 0 or (shi.min() >= 0 and shi.max() < SPLIT)
            chunks.append((slo, dlo, shi, dhi))
        per_core.append(chunks)

    def wrap_idx(flat):
        n = len(flat)
        cols = n // 16
        a = flat.reshape(cols, 16).T.astype(np.int16)      # [16, cols]
        return np.tile(a, (8, 1))                          # [128, cols]

    arrays = []
    for c in range(NCORES):
        idx_cols = []
        dlc = np.full((P, totT), -1.0, np.float16)
        t0 = 0
        for j in range(NCHUNK):
            slo, dlo, shi, dhi = per_core[c][j]
            for (s_j, d_j, T) in ((slo, dlo, T_lo[j]), (shi, dhi, T_hi[j])):
                nslot = int(T) * P
                if nslot == 0:
                    continue
                idx = np.zeros(nslot, np.int16)
                dl = np.full(nslot, -1.0, np.float32)
                idx[: len(s_j)] = s_j
                dl[: len(s_j)] = d_j
                idx_cols.append(wrap_idx(idx))
                dlc[:, t0 : t0 + int(T)] = dl.reshape(int(T), P).T.astype(np.float16)
                t0 += int(T)
        assert t0 == totT
        IDX = np.concatenate(idx_cols, axis=1)
        assert IDX.shape == (P, 8 * totT)
        arrays.append((IDX, dlc))

    plan = {"T_lo": T_lo.tolist(), "T_hi": T_hi.tolist(), "totT": totT,
            "TMAX": TMAX}
    return plan, arrays


# ----------------------------------------------------------------------------
# Bass program
# ----------------------------------------------------------------------------

def build_nc(plan, phases='full', reps=1, max_chunks=None):
    import concourse.bacc as bacc
    import concourse.bass as bass
    import concourse.tile as tile
    from concourse import mybir

    FP16 = mybir.dt.float16
    F32 = mybir.dt.float32
    I16 = mybir.dt.int16
    ALU = mybir.AluOpType
    ACTF = mybir.ActivationFunctionType

    T_lo, T_hi, totT = plan["T_lo"], plan["T_hi"], plan["totT"]
    TMAX = plan["TMAX"]
    NTILE = (N + P - 1) // P            # 391 (last 80 rows)
    WCOLS = HD + 8

    nc = bacc.Bacc("TRN2", target_bir_lowering=False, debug=False,
                   num_devices=NCORES)

    dp = lambda name, shape, dt: nc.declare_dram_parameter(name, shape, dt, isOutput=False)
    FEATT = dp("FEATT", [P, N], FP16)
    FOWNT = dp("FOWNT", [P, NSHARD], FP16)
    IDX = dp("IDX", [P, 8 * totT], I16)
    DLC = dp("DLC", [P, totT], FP16)
    IOTAW = dp("IOTAW", [P, P * TMAX], FP16)
    IDENT = dp("IDENT", [P, P], FP16)
    IDENTF = dp("IDENTF", [P, P], F32)
    W1EXT = dp("W1EXT", [F_IN, WCOLS], FP16)
    W2EXT = dp("W2EXT", [D, WCOLS], FP16)
    B1Q = dp("B1Q", [P, HD], F32)
    B2Q = dp("B2Q", [P, HD], F32)
    WM1F = dp("WM1F", [D, MLP_H], F32)
    BM1 = dp("BM1", [P, MLP_H], F32)
    WM2C1F = dp("WM2C1F", [P, NCLS], F32)
    WM2C2F = dp("WM2C2F", [MLP_H - P, NCLS], F32)
    GB = dp("GB", [P, 4], F32)
    BM2 = dp("BM2", [1, NCLS], F32)
    ONESF = dp("ONESF", [P, 1], F32)

    out = nc.declare_dram_parameter("out", [NSHARD, NCLS], F32, isOutput=True)

    table1 = nc.dram_tensor("table1", [N, ROW], FP16)
    table2 = nc.dram_tensor("table2", [N, ROW], FP16)
    AGSPL = 25 * P                       # AllGather stage split (3200)
    x2sliceA = nc.dram_tensor("x2sliceA", [D, AGSPL], FP16)
    x2sliceB = nc.dram_tensor("x2sliceB", [D, NSHARD - AGSPL], FP16)
    x2fullA = nc.dram_tensor("x2fullA", [NCORES * D, AGSPL], FP16,
                             addr_space="Shared")
    x2fullB = nc.dram_tensor("x2fullB", [NCORES * D, NSHARD - AGSPL], FP16,
                             addr_space="Shared")
    ccin = nc.dram_tensor("ccin", [P, 4], F32)
    ccout = nc.dram_tensor("ccout", [P, 4], F32, addr_space="Shared")

    class _SkipRest(Exception):
        pass

    with tile.TileContext(nc) as tc:
        import contextlib
        try:
          with contextlib.ExitStack() as ctx:
            singles = ctx.enter_context(tc.tile_pool(name="singles", bufs=1))

            def load_const(param, shape, dtype, tag):
                t = singles.tile(shape, dtype, tag=tag)
                nc.sync.dma_start(out=t[:], in_=param[:])
                return t

            identb = load_const(IDENT, [P, P], FP16, "c_ident")
            identf = load_const(IDENTF, [P, P], F32, "c_identf")
            iotaw = load_const(IOTAW, [P, P, TMAX], FP16, "c_iotaw")
            w1ext = load_const(W1EXT, [F_IN, WCOLS], FP16, "c_w1ext")
            w2ext = load_const(W2EXT, [D, WCOLS], FP16, "c_w2ext")
            b1q = load_const(B1Q, [P, HD], F32, "c_b1q")
            b2q = load_const(B2Q, [P, HD], F32, "c_b2q")
            wm1f = load_const(WM1F, [D, MLP_H], F32, "c_wm1f")
            bm1 = load_const(BM1, [P, MLP_H], F32, "c_bm1")
            wm2c1f = load_const(WM2C1F, [P, NCLS], F32, "c_wm2c1f")
            wm2c2f = load_const(WM2C2F, [MLP_H - P, NCLS], F32, "c_wm2c2f")
            gb = load_const(GB, [P, 4], F32, "c_gb")
            bm2 = load_const(BM2, [1, NCLS], F32, "c_bm2")
            onesf = load_const(ONESF, [P, 1], F32, "c_onesf")
            fownt = load_const(FOWNT, [P, NSHARD], FP16, "c_fownt")
            idx_sb = load_const(IDX, [P, 8 * totT], I16, "c_idx")
            dlc_sb = load_const(DLC, [P, totT], FP16, "c_dlc")

            x2t_sb = singles.tile([D, NSHARD], FP16, tag="c_x2t")
            x3t_sb = singles.tile([D, NSHARD], F32, tag="c_x3t")
            erown1 = singles.tile([P, NCHUNK, 4], FP16, tag="c_erown1")
            erown2 = singles.tile([P, NCHUNK, 4], FP16, tag="c_erown2")
            zstore = singles.tile([P, NCHUNK, MLP_H + 1], F32, tag="c_zstore")

            def _run_once():
                nc.vector.memset(erown1[:], 0.0)
                nc.vector.memset(erown2[:], 0.0)
                nc.vector.memset(zstore[:], 0.0)
                # ones column (col MLP_H) for the pass-D folded-constant row
                nc.vector.memset(zstore[:, :, MLP_H:MLP_H + 1], 1.0)

                # ---------------- projection phase (full table) --------------
                def projection(layer):
                    """layer 1: featT -> table1; layer 2: x2fullT -> table2."""
                    F = F_IN if layer == 1 else D
                    wext = w1ext if layer == 1 else w2ext
                    table = table1 if layer == 1 else table2
                    SLAB = 8                     # tiles per load slab
                    BST = 8                      # tiles per store batch
                    with tc.tile_pool(name="proj_sb", bufs=3) as sb, \
                         tc.tile_pool(name="proj_st", bufs=2) as stp, \
                         tc.tile_pool(name="proj_ps", bufs=4, space="PSUM") as ps:
                        nslab = (NTILE + SLAB - 1) // SLAB
                        cast_i = 0
                        slab_order = list(range(nslab))
                        if layer == 2:
                            # A-only slabs first: they depend only on the
                            # stage-A AllGather and overlap the in-flight
                            # stage-B collective.
                            def _a_only(s):
                                r0 = s * SLAB * P
                                hi = min(r0 + SLAB * P, N)
                                for r in (r0, hi - 1):
                                    if (r % NSHARD) >= AGSPL:
                                        return False
                                return (r0 // NSHARD) == ((hi - 1) // NSHARD)
                            slab_order.sort(key=lambda s: not _a_only(s))
                        for s in slab_order:
                            r0 = s * SLAB * P
                            ncols = min(SLAB * P, N - r0)
                            slab = sb.tile([F, SLAB * P], FP16, tag="slab")
                            if layer == 1:
                                nc.sync.dma_start(out=slab[:, 0:ncols],
                                                  in_=FEATT[:, r0:r0 + ncols])
                            else:
                                # x2full rows live in per-core 64-row bands,
                                # split at AGSPL into the A/B staged tensors
                                lo = r0
                                while lo < r0 + ncols:
                                    c = lo // NSHARD
                                    lc = lo - c * NSHARD
                                    if lc < AGSPL:
                                        hi = min(r0 + ncols,
                                                 c * NSHARD + AGSPL)
                                        srct, off = x2fullA, lc
                                    else:
                                        hi = min(r0 + ncols, (c + 1) * NSHARD)
                                        srct, off = x2fullB, lc - AGSPL
                                    nc.sync.dma_start(
                                        out=slab[:, lo - r0:hi - r0],
                                        in_=srct[c * D:(c + 1) * D,
                                                 off:off + (hi - lo)])
                                    lo = hi
                            ntile_s = (ncols + P - 1) // P
                            for b0 in range(0, ntile_s, BST):
                                nb = min(BST, ntile_s - b0)
                                rowt = stp.tile([P, BST, WCOLS], FP16, tag="rowt")
                                for q in range(nb):
                                    k = b0 + q
                                    rows = min(P, ncols - k * P)
                                    hp = ps.tile([P, WCOLS], F32, tag="hp")
                                    nc.tensor.matmul(hp[:rows, :],
                                                     lhsT=slab[:, k * P:k * P + rows],
                                                     rhs=wext[:],
                                                     start=True, stop=True)
                                    # GPSIMD cannot read PSUM: rotate
                                    # 1:2 DVE:Act (DVE is the busier engine)
                                    eng = (nc.vector, nc.scalar,
                                           nc.scalar)[cast_i % 3]
                                    cast_i += 1
                                    if eng is nc.scalar:
                                        nc.scalar.activation(rowt[:rows, q, :],
                                                             hp[:rows, :], ACTF.Copy)
                                    else:
                                        nc.vector.tensor_copy(out=rowt[:rows, q, :],
                                                              in_=hp[:rows, :])
                                rows_b = min(BST * P, ncols - b0 * P)
                                nfull = rows_b // P
                                dst_r0 = r0 + b0 * P
                                if nfull:
                                    trows = table[dst_r0:dst_r0 + nfull * P, 0:WCOLS]
                                    nc.sync.dma_start(
                                        out=trows.rearrange("(q p) c -> p q c", p=P),
                                        in_=rowt[:, 0:nfull, :])
                                tail = rows_b - nfull * P
                                if tail:
                                    nc.sync.dma_start(
                                        out=table[dst_r0 + nfull * P:
                                                  dst_r0 + rows_b, 0:WCOLS],
                                        in_=rowt[:tail, nfull, :])

                # --------------- own-er prologue (per-chunk er) --------------
                def er_prologue(xt_src, wext, dest):
                    with tc.tile_pool(name="er_sb", bufs=2) as sb, \
                         tc.tile_pool(name="er_ps", bufs=1, space="PSUM") as ps:
                        erp_all = ps.tile([P, NCHUNK, 4], F32, tag="erp_all")
                        for j in range(NCHUNK):
                            rows = min(P, NSHARD - j * P)
                            nc.tensor.matmul(erp_all[:rows, j, :],
                                             lhsT=xt_src[:, j * P:j * P + rows],
                                             rhs=wext[:, WCOLS - 4:WCOLS],
                                             start=True, stop=True)
                        nfull = NCHUNK - 1
                        nc.vector.tensor_copy(out=dest[:, 0:nfull, :],
                                              in_=erp_all[:, 0:nfull, :])
                        lrows = NSHARD - (NCHUNK - 1) * P
                        nc.vector.tensor_copy(out=dest[:lrows, nfull, :],
                                              in_=erp_all[:lrows, nfull, :])

                # ------------------------- edge phase ------------------------
                def edge_phase(table, ero, bias_c, layer):
                    nch = NCHUNK if max_chunks is None else min(max_chunks, NCHUNK)
                    with tc.tile_pool(name="eg", bufs=3) as eg, \
                         tc.tile_pool(name="em", bufs=2) as em, \
                         tc.tile_pool(name="emt", bufs=2) as emt, \
                         tc.tile_pool(name="es", bufs=3) as es_pool, \
                         tc.tile_pool(name="eps", bufs=1, space="PSUM") as eps, \
                         tc.tile_pool(name="epa", bufs=2, space="PSUM") as epa, \
                         tc.tile_pool(name="epe", bufs=2, space="PSUM") as epe, \
                         tc.tile_pool(name="epx", bufs=1, space="PSUM") as epx:
                        toff = 0
                        for j in range(nch):
                            Tl, Th = T_lo[j], T_hi[j]
                            T = Tl + Th
                            rows = min(P, NSHARD - j * P)
                            gbuf = eg.tile([P, TMAX, ROW], FP16, tag="gbuf")
                            nc.gpsimd.dma_gather(
                                out_ap=gbuf[:, 0:Tl, :], in_ap=table[0:SPLIT, :],
                                idxs_ap=idx_sb[:, 8 * toff:8 * (toff + Tl)],
                                num_idxs=P * Tl, num_idxs_reg=P * Tl,
                                elem_size=ROW, single_packet=False)
                            if Th:
                                nc.gpsimd.dma_gather(
                                    out_ap=gbuf[:, Tl:T, :],
                                    in_ap=table[N - SPLIT:N, :],
                                    idxs_ap=idx_sb[:, 8 * (toff + Tl):8 * (toff + T)],
                                    num_idxs=P * Th, num_idxs_reg=P * Th,
                                    elem_size=ROW, single_packet=False)
                            # dst mask m_all[p, c, t] = (c == dlc[p, t])
                            m_all = em.tile([P, P, TMAX], FP16, tag="m_all")
                            dsl = dlc_sb[:, toff:toff + T]
                            dlc_b = bass.AP(tensor=dsl.tensor, offset=dsl.offset,
                                            ap=[dsl.ap[0], [0, P]] + dsl.ap[1:])
                            nc.vector.tensor_tensor(out=m_all[:, :, 0:T],
                                                    in0=iotaw[:, :, 0:T],
                                                    in1=dlc_b, op=ALU.is_equal)
                            # mt = transpose(m_all) per tile, via PE + one copy
                            mtp = eps.tile([P, TMAX, P], FP16, tag="mtp")
                            for t in range(T):
                                nc.tensor.transpose(out=mtp[:, t, :],
                                                    in_=m_all[:, :, t],
                                                    identity=identb[:])
                            mt = emt.tile([P, TMAX, P], FP16, tag="mt")
                            nc.scalar.activation(mt[:, 0:T, :], mtp[:, 0:T, :],
                                                 ACTF.Copy)
                            # er per slot: erp[p, t, :] = mt_t^T @ ero
                            erp = epe.tile([P, TMAX, 4], F32, tag="erp")
                            for t in range(T):
                                nc.tensor.matmul(erp[:, t, :], lhsT=mt[:, t, :],
                                                 rhs=ero[:, j, :],
                                                 start=True, stop=True)
                            # e = el + er ; lrelu ; exp -> gbuf[...,260:264]
                            e_sb = es_pool.tile([P, TMAX, 4], F32, tag="e_sb")
                            nc.vector.tensor_tensor(out=e_sb[:, 0:T, :],
                                                    in0=gbuf[:, 0:T, ELOFF:ELOFF + 4],
                                                    in1=erp[:, 0:T, :], op=ALU.add)
                            lr = es_pool.tile([P, TMAX, 4], F32, tag="lr")
                            nc.scalar.activation(lr[:, 0:T, :], e_sb[:, 0:T, :],
                                                 ACTF.Prelu, alpha=NEG)
                            gex = gbuf[:, 0, EROFF:EROFF + 4]
                            ex_out = bass.AP(tensor=gex.tensor, offset=gex.offset,
                                             ap=[gex.ap[0], [ROW, T], [1, 4]])
                            nc.scalar.activation(ex_out, lr[:, 0:T, :], ACTF.Exp)
                            # h *= ex (2x-rate thanks to (d,h) interleave)
                            gb0 = gbuf[:, 0, 0:HD]
                            hv = bass.AP(tensor=gb0.tensor, offset=gb0.offset,
                                         ap=[gb0.ap[0], [ROW, T], [H, D], [1, H]])
                            ex_b = bass.AP(tensor=gex.tensor, offset=gex.offset,
                                           ap=[gex.ap[0], [ROW, T], [0, D], [1, H]])
                            nc.vector.tensor_tensor(out=hv, in0=hv, in1=ex_b,
                                                    op=ALU.mult)
                            # aggregate: T accumulating mask matmuls
                            agg = epa.tile([P, WCOLS], F32, tag="agg")
                            for t in range(T):
                                nc.tensor.matmul(agg[:], lhsT=m_all[:, :, t],
                                                 rhs=gbuf[:, t, 0:WCOLS],
                                                 start=(t == 0), stop=(t == T - 1))
                            # node-space epilogue on DVE(recip) + GpSimd
                            sr = es_pool.tile([P, 4], F32, tag="sr")
                            nc.vector.reciprocal(sr[:], agg[:, EROFF:EROFF + 4])
                            agg_r = agg[:, 0:HD].rearrange("p (d h) -> p d h", h=H)
                            sr_ap = sr[:]
                            sr_b = bass.AP(tensor=sr_ap.tensor, offset=sr_ap.offset,
                                           ap=[sr_ap.ap[0], [0, D], [1, H]])
                            osb = es_pool.tile([P, D, H], F32, tag="osb")
                            # agg is PSUM: this one stays on DVE (GPSIMD
                            # cannot read PSUM); the rest go to GpSimd.
                            nc.vector.tensor_tensor(out=osb[:], in0=agg_r,
                                                    in1=sr_b, op=ALU.mult)
                            bias_r = bias_c[:].rearrange("p (d h) -> p d h", h=H)
                            nc.vector.tensor_tensor(out=osb[:], in0=osb[:],
                                                    in1=bias_r, op=ALU.add)
                            # 0.25*relu(x) == relu(0.25*x): head-mean scale
                            # folded into the Act scale.
                            nc.scalar.activation(osb[:], osb[:], ACTF.Relu,
                                                 scale=0.25)
                            xo = es_pool.tile([P, D], F32, tag="xo")
                            nc.vector.tensor_reduce(
                                out=xo[:], in_=osb[:],
                                axis=mybir.AxisListType.X, op=ALU.add)
                            # transpose -> [D, rows] and store into xT slab
                            xop = epx.tile([D, P], F32, tag="xop")
                            nc.tensor.transpose(out=xop[:, 0:rows],
                                                in_=xo[:rows, :],
                                                identity=identf[:rows, :rows])
                            if layer == 1:
                                nc.scalar.activation(
                                    x2t_sb[:, j * P:j * P + rows],
                                    xop[:, 0:rows], ACTF.Copy)
                            else:
                                nc.scalar.activation(
                                    x3t_sb[:, j * P:j * P + rows],
                                    xop[:, 0:rows], ACTF.Copy)
                            if layer == 1 and j == 24:
                                nc.sync.dma_start(out=x2sliceA[:],
                                                  in_=x2t_sb[:, 0:AGSPL])
                                nc.gpsimd.collective_compute(
                                    "AllGather", mybir.AluOpType.bypass,
                                    replica_groups=[list(range(NCORES))],
                                    ins=[x2sliceA[:]], outs=[x2fullA[:]])
                            toff += T

                # ------------------------------ go ---------------------------
                order = ["P1", "E1", "AG", "P2", "E2", "full"]
                upto = order.index(phases)
                done = False

                projection(1)
                er_prologue(fownt, w1ext, erown1)
                done = upto <= order.index("P1")
                if not done:
                    edge_phase(table1, erown1, b1q, layer=1)
                    nc.sync.dma_start(out=x2sliceB[:],
                                      in_=x2t_sb[:, AGSPL:NSHARD])
                    done = upto <= order.index("E1")
                if not done:
                    nc.gpsimd.collective_compute(
                        "AllGather", mybir.AluOpType.bypass,
                        replica_groups=[list(range(NCORES))],
                        ins=[x2sliceB[:]], outs=[x2fullB[:]])
                    done = upto <= order.index("AG")
                if not done:
                    projection(2)
                    er_prologue(x2t_sb, w2ext, erown2)
                    done = upto <= order.index("P2")
                if not done:
                    edge_phase(table2, erown2, b2q, layer=2)
                    done = upto <= order.index("E2")
                if done:
                    with tc.tile_pool(name="dbg0", bufs=1) as dbg0:
                        z = dbg0.tile([P, NCLS], F32, tag="dbgz")
                        nc.vector.memset(z[:], 0.0)
                        for j in range(NCHUNK):
                            r0 = j * P
                            rows = min(P, NSHARD - r0)
                            nc.sync.dma_start(out=out[r0:r0 + rows, :], in_=z[:rows])
                    raise _SkipRest()

                # ------------------------------ MLP --------------------------
                # pass A: z = relu(x3 @ Wm1 + bm1) -> zstore; raw moment sums
                with tc.tile_pool(name="ma", bufs=3) as ma, \
                     tc.tile_pool(name="map", bufs=2, space="PSUM") as map_, \
                     tc.tile_pool(name="sta", bufs=1, space="PSUM") as sta:
                    sa1 = sta.tile([P, 1], F32, tag="sa1")
                    sa2 = sta.tile([P, 1], F32, tag="sa2")
                    sq1 = sta.tile([P, 1], F32, tag="sq1")
                    sq2 = sta.tile([P, 1], F32, tag="sq2")
                    for j in range(NCHUNK):
                        rows = min(P, NSHARD - j * P)
                        zp = map_.tile([P, MLP_H], F32, tag="zp")
                        nc.tensor.matmul(zp[:rows, :],
                                         lhsT=x3t_sb[:, j * P:j * P + rows],
                                         rhs=wm1f[:], start=True, stop=True)
                        zc = zstore[:, j, 0:MLP_H]
                        nc.vector.tensor_tensor(out=zc[:rows], in0=zp[:rows],
                                                in1=bm1[:rows], op=ALU.add)
                        nc.scalar.activation(zc[:rows], zc[:rows], ACTF.Relu)
                        zq = ma.tile([P, MLP_H], F32, tag="zq")
                        nc.vector.tensor_tensor(out=zq[:rows], in0=zc[:rows],
                                                in1=zc[:rows], op=ALU.mult)
                        first, last = (j == 0), (j == NCHUNK - 1)
                        nc.tensor.matmul(sa1[:], lhsT=zc[:rows, 0:P],
                                         rhs=onesf[:rows], start=first, stop=last)
                        nc.tensor.matmul(sa2[:MLP_H - P], lhsT=zc[:rows, P:MLP_H],
                                         rhs=onesf[:rows], start=first, stop=last)
                        nc.tensor.matmul(sq1[:], lhsT=zq[:rows, 0:P],
                                         rhs=onesf[:rows], start=first, stop=last)
                        nc.tensor.matmul(sq2[:MLP_H - P], lhsT=zq[:rows, P:MLP_H],
                                         rhs=onesf[:rows], start=first, stop=last)
                    pk = ma.tile([P, 4], F32, tag="pk")
                    nc.vector.memset(pk[:], 0.0)
                    nc.vector.tensor_copy(out=pk[:, 0:1], in_=sa1[:])
                    nc.vector.tensor_copy(out=pk[:MLP_H - P, 1:2], in_=sa2[:MLP_H - P])
                    nc.vector.tensor_copy(out=pk[:, 2:3], in_=sq1[:])
                    nc.vector.tensor_copy(out=pk[:MLP_H - P, 3:4], in_=sq2[:MLP_H - P])
                    nc.sync.dma_start(out=ccin[:], in_=pk[:])

                nc.gpsimd.collective_compute(
                    "AllReduce", mybir.AluOpType.add,
                    replica_groups=[list(range(NCORES))],
                    ins=[ccin[:]], outs=[ccout[:]])

                # pass C: BN constants folded into final weights
                with tc.tile_pool(name="m2", bufs=3) as m2, \
                     tc.tile_pool(name="m2p", bufs=2, space="PSUM") as m2p:
                    stg = m2.tile([P, 4], F32, tag="stg")
                    nc.sync.dma_start(out=stg[:], in_=ccout[:])
                    m1t = m2.tile([P, 2], F32, tag="m1t")
                    nc.vector.tensor_scalar(out=m1t[:], in0=stg[:, 0:2],
                                            scalar1=1.0 / N, scalar2=None,
                                            op0=ALU.mult)
                    m2t = m2.tile([P, 2], F32, tag="m2t")
                    nc.vector.tensor_scalar(out=m2t[:], in0=stg[:, 2:4],
                                            scalar1=1.0 / N, scalar2=None,
                                            op0=ALU.mult)
                    var = m2.tile([P, 2], F32, tag="var")
                    nc.vector.tensor_tensor(out=var[:], in0=m1t[:], in1=m1t[:],
                                            op=ALU.mult)
                    nc.vector.tensor_tensor(out=var[:], in0=m2t[:], in1=var[:],
                                            op=ALU.subtract)
                    nc.vector.tensor_scalar(out=var[:], in0=var[:], scalar1=EPS,
                                            scalar2=None, op0=ALU.add)
                    std = m2.tile([P, 2], F32, tag="std")
                    nc.scalar.activation(std[:], var[:], ACTF.Sqrt)
                    rstd = m2.tile([P, 2], F32, tag="rstd")
                    nc.vector.reciprocal(rstd[:], std[:])
                    gp = m2.tile([P, 2], F32, tag="gp")
                    nc.vector.tensor_tensor(out=gp[:], in0=gb[:, 0:2], in1=rstd[:],
                                            op=ALU.mult)
                    bp = m2.tile([P, 2], F32, tag="bp")
                    nc.vector.tensor_tensor(out=bp[:], in0=m1t[:], in1=gp[:],
                                            op=ALU.mult)
                    nc.vector.tensor_tensor(out=bp[:], in0=gb[:, 2:4], in1=bp[:],
                                            op=ALU.subtract)
                    wp1 = m2.tile([P, NCLS], F32, tag="wp1")
                    nc.vector.tensor_scalar_mul(wp1[:], wm2c1f[:], gp[:, 0:1])
                    wp2 = m2.tile([P, NCLS], F32, tag="wp2")
                    nc.vector.memset(wp2[:], 0.0)
                    nc.vector.tensor_scalar_mul(wp2[:MLP_H - P, :], wm2c2f[:],
                                                gp[:MLP_H - P, 1:2])
                    cp = m2p.tile([1, NCLS], F32, tag="cp")
                    nc.tensor.matmul(cp[:], lhsT=bp[:, 0:1], rhs=wm2c1f[:],
                                     start=True, stop=False)
                    nc.tensor.matmul(cp[:], lhsT=bp[:MLP_H - P, 1:2], rhs=wm2c2f[:],
                                     start=False, stop=True)
                    cps = m2.tile([1, NCLS], F32, tag="cps")
                    nc.vector.tensor_tensor(out=cps[:], in0=cp[:], in1=bm2[:],
                                            op=ALU.add)
                    # place c'' into wp2 row (MLP_H - P) — pairs with the
                    # all-ones zstore column MLP_H. DMA: cross-partition move.
                    nc.sync.dma_start(out=wp2[MLP_H - P:MLP_H - P + 1, :],
                                      in_=cps[:])

                    # pass D: out = z @ W'' (+ c'' via ones column)
                    for j in range(NCHUNK):
                        r0 = j * P
                        rows = min(P, NSHARD - r0)
                        t1 = m2p.tile([P, P], F32, tag="t1")
                        nc.tensor.transpose(out=t1[:], in_=zstore[:, j, 0:P],
                                            identity=identf[:])
                        zT1 = m2.tile([P, P], F32, tag="zT1")
                        nc.scalar.activation(zT1[:], t1[:], ACTF.Copy)
                        t2 = m2p.tile([MLP_H + 1 - P, P], F32, tag="t2")
                        nc.tensor.transpose(out=t2[:], in_=zstore[:, j, P:MLP_H + 1],
                                            identity=identf[:])
                        zT2 = m2.tile([MLP_H + 1 - P, P], F32, tag="zT2")
                        nc.scalar.activation(zT2[:], t2[:], ACTF.Copy)
                        op_ = m2p.tile([P, NCLS], F32, tag="op")
                        nc.tensor.matmul(op_[:], lhsT=zT1[:], rhs=wp1[:],
                                         start=True, stop=False)
                        nc.tensor.matmul(op_[:], lhsT=zT2[:], rhs=wp2[:MLP_H + 1 - P, :],
                                         start=False, stop=True)
                        ofin = m2.tile([P, NCLS], F32, tag="ofin")
                        nc.vector.tensor_copy(out=ofin[:rows], in_=op_[:rows])
                        nc.sync.dma_start(out=out[r0:r0 + rows, :], in_=ofin[:rows])

            for _rep in range(reps):
                try:
                    _run_once()
                except _SkipRest:
                    pass

        except _SkipRest:
            pass
    nc.finalize()
    return nc


# ----------------------------------------------------------------------------
# Host entry
# ----------------------------------------------------------------------------

def prep_inputs(inputs):
    f32 = np.float32
    perm = _perm()
    W1ext = _fold_weights(np.asarray(inputs["W1"], f32),
                          np.asarray(inputs["al1"], f32),
                          np.asarray(inputs["ar1"], f32))
    W2ext = _fold_weights(np.asarray(inputs["W2"], f32),
                          np.asarray(inputs["al2"], f32),
                          np.asarray(inputs["ar2"], f32))
    plan, earrays = _prep_edges(inputs["src"], inputs["dst"])
    TMAX = plan["TMAX"]

    featT = np.ascontiguousarray(
        np.asarray(inputs["features"], f32).astype(np.float16).T)  # [128, N]
    iotaw = np.broadcast_to(
        np.repeat(np.arange(P, dtype=np.float16), TMAX)[None, :],
        (P, P * TMAX)).copy()
    bc = lambda v: np.broadcast_to(np.asarray(v, f32).reshape(1, -1),
                                   (P, np.asarray(v).size)).copy()
    consts = {
        "IDENT": np.eye(P, dtype=np.float16),
        "IDENTF": np.eye(P, dtype=f32),
        "IOTAW": iotaw,
        "W1EXT": W1ext, "W2EXT": W2ext,
        "B1Q": bc(np.asarray(inputs["b1"], f32)[perm]),
        "B2Q": bc(np.asarray(inputs["b2"], f32)[perm]),
        "WM1F": np.asarray(inputs["Wm1"], f32),
        "BM1": bc(inputs["bm1"]),
        "WM2C1F": np.asarray(inputs["Wm2"], f32)[0:P, :],
        "WM2C2F": np.asarray(inputs["Wm2"], f32)[P:MLP_H, :],
        "BM2": np.asarray(inputs["bm2"], f32).reshape(1, NCLS),
        "ONESF": np.ones((P, 1), f32),
    }
    gbarr = np.zeros((P, 4), f32)
    gamma = np.asarray(inputs["gamma"], f32)
    beta = np.asarray(inputs["beta"], f32)
    gbarr[:, 0] = gamma[0:P]
    gbarr[0:MLP_H - P, 1] = gamma[P:MLP_H]
    gbarr[:, 2] = beta[0:P]
    gbarr[0:MLP_H - P, 3] = beta[P:MLP_H]
    consts["GB"] = gbarr

    in_maps = []
    for c in range(NCORES):
        IDXa, DLCa = earrays[c]
        m = dict(consts)
        m["FEATT"] = featT
        m["FOWNT"] = np.ascontiguousarray(featT[:, c * NSHARD:(c + 1) * NSHARD])
        m["IDX"] = IDXa
        m["DLC"] = DLCa
        in_maps.append(m)
    return plan, in_maps


def kernel(**inputs):
    from concourse.bass_utils import run_bass_kernel_spmd
    plan, in_maps = prep_inputs(inputs)
    nc = build_nc(plan)
    res = run_bass_kernel_spmd(nc, in_maps, core_ids=list(range(NCORES)))
    out = np.concatenate([res.results[c]["out"] for c in range(NCORES)], axis=0)
    return out.astype(np.float32)


if __name__ == "__main__":
    import time
    t0 = time.time()
    plan, _ = _prep_edges(
        np.concatenate([np.random.randint(0, N, 800000), np.arange(N)]).astype(np.int32),
        np.concatenate([np.random.randint(0, N, 800000), np.arange(N)]).astype(np.int32))
    print("edge prep:", time.time() - t0, "totT:", plan["totT"], "TMAX:", plan["TMAX"])
    t0 = time.time()
    nc = build_nc(plan)
    print("build:", time.time() - t0)
